# revision 15
# baseline (speedup 1.0000x reference)
"""Trainium2 Bass kernel for nn_BEMBFlex (within-category log-softmax utility model).

Strategy: shard ITEMS BY CATEGORY across the 8 cores. Categories are
rank-sorted by size and dealt round-robin (rank % 8 -> shard), so every
shard receives the same padded column layout and one SPMD program serves
all cores. Each core computes util = [th|ze] @ [alphaT; item_obsT] for all
1024 sessions over its ~1/8 of the items (lambda folded in during the PSUM
eviction via a DMA-broadcast tile), then does the within-category
log-softmax locally (categories never span shards). Output leaves the
device as fp16 (halves the dominant out-DMA); the host de-permutes the
padded columns and upcasts to f32.
"""

import sys

for _p in ("/opt/trn_rl_repo",):
    if _p not in sys.path:
        sys.path.insert(0, _p)

import ml_dtypes
import numpy as np

import concourse.bass as bass
import concourse.tile as tile
from concourse import bacc, bass_utils, mybir

# Force Exp/Ln/Copy onto the one ACT table set that holds all three
# (natural_log_exp_and_others), so the kernel pays a single table load
# instead of reloading on every Exp<->Ln alternation. Entries keep their
# positions (set ids are positional); other sets merely lose the funcs
# that the combined set provides.
_orig_gat = bacc.get_activation_tables


def _gat_forced(arch):
    t = _orig_gat(arch)
    target = "natural_log_exp_and_others"
    if target in t:
        t = {k: (v if k == target else v - t[target]) for k, v in t.items()}
    return t


bacc.get_activation_tables = _gat_forced

NUM_USERS = 100000
NUM_ITEMS = 25000
NUM_CATS = 500
LATENT = 64
BATCH = 1024
NCORES = 8
P = 128                    # partitions / sessions per matmul chunk
NCHUNKS = BATCH // P       # session chunks per core
BLOCK_COLS = 2048          # max padded cols per processing block (4 PSUM banks)
JGROUP = 2                 # session-chunks sharing one ln batch
PAD_NEG = -1.0e30
LN2 = 0.6931471805599453

F32 = mybir.dt.float32
F16 = mybir.dt.float16
BF16 = mybir.dt.bfloat16
I32 = mybir.dt.int32

_nc_cache = {}


# ----------------------------------------------------------------------------
# Host-side layout
# ----------------------------------------------------------------------------

def _layout(cat_sizes):
    """Slot/block layout shared by all 8 shards.

    Categories sorted by size desc; slot i holds category ranks
    [8i, 8i+8) (one per shard). Slot width L_i = first (largest) size in
    the group rounded up to a multiple of 4. Blocks greedily group
    consecutive slots under a uniform L (the first slot's L) with
    g*L <= BLOCK_COLS.
    """
    order = np.argsort(-cat_sizes, kind="stable")
    order = order[cat_sizes[order] > 0]
    ncats = len(order)
    nslots = -(-ncats // NCORES)
    slot_L = np.empty(nslots, np.int64)
    for i in range(nslots):
        mx = int(cat_sizes[order[i * NCORES]])
        slot_L[i] = max(4, ((mx + 3) // 4) * 4)
    blocks = []  # (col0, g, L, slot0)
    col = 0
    i = 0
    while i < nslots:
        Lb = int(slot_L[i])
        # keep the first block small so the first matmul->evict chain
        # starts as early as possible
        cap = 512 if not blocks else BLOCK_COLS
        g = 1
        while i + g < nslots and (g + 1) * Lb <= cap:
            g += 1
        blocks.append((col, g, Lb, i))
        col += g * Lb
        i += g
    ipad = col
    slot_col = np.empty(nslots, np.int64)
    for (c0, g, Lb, s0) in blocks:
        for q in range(g):
            slot_col[s0 + q] = c0 + q * Lb
    return order, blocks, ipad, slot_col


def _prep(inputs):
    cat = np.asarray(inputs["category_idx"]).astype(np.int64).ravel()
    cat_sizes = np.bincount(cat, minlength=NUM_CATS)
    order, blocks, ipad, slot_col = _layout(cat_sizes)

    rank = np.full(NUM_CATS, -1, np.int64)
    rank[order] = np.arange(len(order))

    # position of each item within its category (stable order)
    perm = np.argsort(cat, kind="stable")
    starts = np.searchsorted(cat[perm], np.arange(NUM_CATS))
    within_sorted = np.arange(NUM_ITEMS) - starts[cat[perm]]
    item_within = np.empty(NUM_ITEMS, np.int64)
    item_within[perm] = within_sorted

    r = rank[cat]
    item_shard = r % NCORES
    item_col = slot_col[r // NCORES] + item_within

    alpha = np.ascontiguousarray(np.asarray(inputs["alpha_item"], np.float32))
    obs = np.ascontiguousarray(np.asarray(inputs["item_obs"], np.float32))
    lam = np.asarray(inputs["lambda_item"], np.float32).ravel()

    W = np.zeros((NCORES, 2 * LATENT, ipad), np.float32)
    LAM = np.full((NCORES, 1, ipad), PAD_NEG, np.float32)
    for s in range(NCORES):
        m = item_shard == s
        cols = item_col[m]
        W[s, 0:LATENT, cols] = alpha[m]
        W[s, LATENT:, cols] = obs[m]
        LAM[s, 0, cols] = lam[m]
    W = W.astype(ml_dtypes.bfloat16)
    LAM = LAM.astype(ml_dtypes.bfloat16)

    uidx = np.asarray(inputs["user_index"]).astype(np.int64).ravel()
    theta = np.asarray(inputs["theta_user"], np.float32)
    zeta = np.asarray(inputs["zeta_user"], np.float32)
    # [2K, B] pre-gathered, pre-transposed session features (host-side shard prep)
    thzet = np.ascontiguousarray(
        np.concatenate([theta[uidx], zeta[uidx]], axis=1).T
    ).astype(ml_dtypes.bfloat16)
    return {
        "blocks": blocks,
        "ipad": ipad,
        "item_shard": item_shard,
        "item_col": item_col,
        "W": W,
        "LAM": LAM,
        "thzet": thzet,
    }


# ----------------------------------------------------------------------------
# Device program
# ----------------------------------------------------------------------------

def _bcast3(t2d, L):
    """[P, g] tile -> [P, g, L] read-AP with step-0 innermost broadcast."""
    ap = t2d[:, :]
    return bass.AP(tensor=ap.tensor, offset=ap.offset, ap=[*ap.ap, [0, L]])


def _build_nc(blocks, ipad):
    nc = bacc.Bacc(
        "TRN2",
        debug=False,
        enable_asserts=False,
        target_bir_lowering=False,
        num_devices=NCORES,
    )
    w_d = nc.dram_tensor("W", [2 * LATENT, ipad], BF16, kind="ExternalInput").ap()
    lam_d = nc.dram_tensor("LAM", [1, ipad], BF16, kind="ExternalInput").ap()
    thzet_d = nc.dram_tensor("THZET", [2 * LATENT, BATCH], BF16, kind="ExternalInput").ap()
    out_d = nc.dram_tensor("O", [BATCH, ipad], F16, kind="ExternalOutput").ap()

    gtot = sum(g for (_c, g, _l, _s) in blocks)
    with tile.TileContext(nc) as tc:
        with (
            tc.tile_pool(name="singles", bufs=1) as singles,
            tc.tile_pool(name="psum_u", bufs=2, space="PSUM") as psum_u,
            tc.tile_pool(name="ubuf", bufs=16) as ubuf,
            tc.tile_pool(name="exbuf", bufs=6) as exbuf,
            tc.tile_pool(name="stats", bufs=6) as stats,
        ):
            # DMA order: first block's W + lambda first, then the first two
            # session chunks, then the rest -- so the first matmul->evict
            # chain starts as early as possible.
            thzet_sb = singles.tile([2 * LATENT, BATCH], BF16, name="thzet_sb")
            w_sb = singles.tile([2 * LATENT, ipad], BF16, name="w_sb")
            lam128 = singles.tile([P, ipad], BF16, name="lam128")

            def load_block(col0, cols):
                nc.sync.dma_start(
                    out=w_sb[:, col0:col0 + cols], in_=w_d[:, col0:col0 + cols]
                )
                nc.sync.dma_start(
                    out=lam128[:, col0:col0 + cols],
                    in_=bass.AP(
                        tensor=lam_d.tensor, offset=col0, ap=[[0, P], [1, cols]]
                    ),
                )

            (c0_0, g_0, L_0, _s) = blocks[0]
            load_block(c0_0, g_0 * L_0)
            nc.sync.dma_start(
                out=thzet_sb[:, 0:2 * P], in_=thzet_d[:, 0:2 * P]
            )
            for (col0, g, L, _s0) in blocks[1:]:
                load_block(col0, g * L)
            nc.sync.dma_start(
                out=thzet_sb[:, 2 * P:], in_=thzet_d[:, 2 * P:]
            )
            thze_t = [thzet_sb[:, j * P:(j + 1) * P] for j in range(NCHUNKS)]

            # session-chunk groups: full JGROUP-sized groups up front, then
            # single-chunk groups at the end so the kernel tail (ln -> final
            # -> DMA of the last group) is as short as possible
            groups = []
            j = 0
            while j < NCHUNKS:
                n = JGROUP if j < NCHUNKS - 2 else 1
                groups.append(list(range(j, j + n)))
                j += n
            for js in groups:
                u1s = {}
                s_g = stats.tile([P, len(js) * gtot], F32, name="s_g", tag="s_g")
                for t, j in enumerate(js):
                    goff = t * gtot
                    for bi, (col0, g, L, _s0) in enumerate(blocks):
                        cols = g * L
                        up = psum_u.tile([P, cols], F32, name="up", tag="up")
                        for c0 in range(0, cols, 512):
                            cn = min(512, cols - c0)
                            nc.tensor.matmul(
                                up[:, c0:c0 + cn],
                                lhsT=thze_t[j],
                                rhs=w_sb[:, col0 + c0:col0 + c0 + cn],
                                start=True,
                                stop=True,
                            )
                        # evict PSUM: u1 = u + lambda  (no exp-shift needed:
                        # |u1| < ~90 so exp stays inside f32 range, and ln(s)
                        # is computed exactly via exponent extraction)
                        u1 = ubuf.tile([P, cols], F32, name="u1", tag="u1")
                        nc.vector.scalar_tensor_tensor(
                            out=u1[:, :],
                            in0=up[:, :],
                            scalar=0.0,
                            in1=lam128[:, col0:col0 + cols],
                            op0=mybir.AluOpType.add,
                            op1=mybir.AluOpType.add,
                        )
                        u1s[(t, bi)] = u1
                        ex = exbuf.tile([P, cols], BF16, name="ex", tag="ex")
                        nc.scalar.activation(
                            out=ex[:, :], in_=u1[:, :],
                            func=mybir.ActivationFunctionType.Exp,
                        )
                        nc.vector.reduce_sum(
                            out=s_g[:, goff:goff + g],
                            in_=ex[:, :].rearrange("p (g l) -> p g l", l=L),
                            axis=mybir.AxisListType.X,
                        )
                        goff += g
                # ls = ln(s) via exponent extraction: s = 2^e * m, m in [1,2):
                # ln(s) = Ln_table(m) + (e-127)*ln2. Keeps the Ln table input
                # in its accurate range regardless of s's magnitude.
                gw = len(js) * gtot
                si = s_g[:, :].bitcast(mybir.dt.int32)
                eexp = stats.tile([P, gw], mybir.dt.int32, name="eexp", tag="eexp")
                nc.vector.tensor_scalar(
                    out=eexp[:, :], in0=si, scalar1=23, scalar2=None,
                    op0=mybir.AluOpType.logical_shift_right,
                )
                mbits = stats.tile([P, gw], mybir.dt.int32, name="mbits", tag="mbits")
                nc.vector.tensor_scalar(
                    out=mbits[:, :], in0=si,
                    scalar1=0x007FFFFF, scalar2=0x3F800000,
                    op0=mybir.AluOpType.bitwise_and,
                    op1=mybir.AluOpType.bitwise_or,
                )
                # e2 = (e - 127) * ln2 on ACT (Copy: out = in*scale + bias)
                e2 = stats.tile([P, gw], F32, name="e2", tag="e2")
                nc.scalar.activation(
                    out=e2[:, :], in_=eexp[:, :],
                    func=mybir.ActivationFunctionType.Copy,
                    bias=float(-127.0 * LN2),
                    scale=float(LN2),
                )
                lnm = stats.tile([P, gw], F32, name="lnm", tag="lnm")
                nc.scalar.activation(
                    out=lnm[:, :], in_=mbits[:, :].bitcast(F32),
                    func=mybir.ActivationFunctionType.Ln,
                )
                ls_g = stats.tile([P, gw], F32, name="ls_g", tag="ls_g")
                nc.vector.tensor_add(out=ls_g[:, :], in0=lnm[:, :], in1=e2[:, :])
                for t, j in enumerate(js):
                    goff = t * gtot
                    feng = nc.gpsimd
                    for bi, (col0, g, L, _s0) in enumerate(blocks):
                        cols = g * L
                        u1 = u1s[(t, bi)]
                        ls_b = _bcast3(ls_g[:, goff:goff + g], L)
                        # fp16 result written in place over u1's low half:
                        # out elem i lands behind the (monotonic) read cursor
                        # of in elem i, so the overlap is safe.
                        o16 = u1[:, :].bitcast(F16)[:, 0:cols]
                        feng.tensor_tensor(
                            out=o16.rearrange("p (g l) -> p g l", l=L),
                            in0=u1[:, :].rearrange("p (g l) -> p g l", l=L),
                            in1=ls_b,
                            op=mybir.AluOpType.subtract,
                        )
                        nc.sync.dma_start(
                            out=out_d[j * P:(j + 1) * P, col0:col0 + cols],
                            in_=o16,
                        )
                        goff += g
    nc.compile()
    return nc


# ----------------------------------------------------------------------------
# Entry points
# ----------------------------------------------------------------------------

def run(inputs, trace=False):
    prep = _prep(inputs)
    key = (prep["ipad"], tuple(prep["blocks"]))
    nc = _nc_cache.get(key)
    if nc is None:
        nc = _build_nc(prep["blocks"], prep["ipad"])
        _nc_cache[key] = nc
    in_maps = [
        {
            "W": prep["W"][c],
            "LAM": prep["LAM"][c],
            "THZET": prep["thzet"],
        }
        for c in range(NCORES)
    ]
    res = bass_utils.run_bass_kernel_spmd(
        nc, in_maps, core_ids=list(range(NCORES)), trace=trace
    )
    big = np.stack([res.results[c]["O"] for c in range(NCORES)])  # [8, B, ipad] f16
    out = np.ascontiguousarray(
        big[prep["item_shard"], :, prep["item_col"]].T
    ).astype(np.float32)
    return out, res


def kernel(**inputs) -> np.ndarray:
    out, _ = run(inputs, trace=False)
    return out


# revision 16
# speedup vs baseline: 1.0116x; 1.0116x over previous
"""Trainium2 Bass kernel for nn_BEMBFlex (within-category log-softmax utility model).

Strategy: shard ITEMS BY CATEGORY across the 8 cores. Categories are
rank-sorted by size and dealt round-robin (rank % 8 -> shard), so every
shard receives the same padded column layout and one SPMD program serves
all cores. Each core computes util = [th|ze] @ [alphaT; item_obsT] for all
1024 sessions over its ~1/8 of the items (lambda folded in during the PSUM
eviction via a DMA-broadcast tile), then does the within-category
log-softmax locally (categories never span shards). Output leaves the
device as fp16 (halves the dominant out-DMA); the host de-permutes the
padded columns and upcasts to f32.
"""

import sys

for _p in ("/opt/trn_rl_repo",):
    if _p not in sys.path:
        sys.path.insert(0, _p)

import ml_dtypes
import numpy as np

import concourse.bass as bass
import concourse.tile as tile
from concourse import bacc, bass_utils, mybir

# Force Exp/Ln/Copy onto the one ACT table set that holds all three
# (natural_log_exp_and_others), so the kernel pays a single table load
# instead of reloading on every Exp<->Ln alternation. Entries keep their
# positions (set ids are positional); other sets merely lose the funcs
# that the combined set provides.
_orig_gat = bacc.get_activation_tables


def _gat_forced(arch):
    t = _orig_gat(arch)
    target = "natural_log_exp_and_others"
    if target in t:
        t = {k: (v if k == target else v - t[target]) for k, v in t.items()}
    return t


bacc.get_activation_tables = _gat_forced

NUM_USERS = 100000
NUM_ITEMS = 25000
NUM_CATS = 500
LATENT = 64
BATCH = 1024
NCORES = 8
P = 128                    # partitions / sessions per matmul chunk
NCHUNKS = BATCH // P       # session chunks per core
BLOCK_COLS = 2048          # max padded cols per processing block (4 PSUM banks)
JGROUP = 2                 # session-chunks sharing one ln batch
PAD_NEG = -1.0e30
LN2 = 0.6931471805599453

F32 = mybir.dt.float32
F16 = mybir.dt.float16
BF16 = mybir.dt.bfloat16
I32 = mybir.dt.int32

_nc_cache = {}


# ----------------------------------------------------------------------------
# Host-side layout
# ----------------------------------------------------------------------------

def _layout(cat_sizes):
    """Slot/block layout shared by all 8 shards.

    Categories sorted by size desc; slot i holds category ranks
    [8i, 8i+8) (one per shard). Slot width L_i = first (largest) size in
    the group rounded up to a multiple of 4. Blocks greedily group
    consecutive slots under a uniform L (the first slot's L) with
    g*L <= BLOCK_COLS.
    """
    order = np.argsort(-cat_sizes, kind="stable")
    order = order[cat_sizes[order] > 0]
    ncats = len(order)
    nslots = -(-ncats // NCORES)
    slot_L = np.empty(nslots, np.int64)
    for i in range(nslots):
        mx = int(cat_sizes[order[i * NCORES]])
        slot_L[i] = max(4, ((mx + 3) // 4) * 4)
    blocks = []  # (col0, g, L, slot0)
    col = 0
    i = 0
    while i < nslots:
        Lb = int(slot_L[i])
        # keep the first block small so the first matmul->evict chain
        # starts as early as possible
        cap = 512 if not blocks else BLOCK_COLS
        g = 1
        while i + g < nslots and (g + 1) * Lb <= cap:
            g += 1
        blocks.append((col, g, Lb, i))
        col += g * Lb
        i += g
    ipad = col
    slot_col = np.empty(nslots, np.int64)
    for (c0, g, Lb, s0) in blocks:
        for q in range(g):
            slot_col[s0 + q] = c0 + q * Lb
    return order, blocks, ipad, slot_col


def _prep(inputs):
    cat = np.asarray(inputs["category_idx"]).astype(np.int64).ravel()
    cat_sizes = np.bincount(cat, minlength=NUM_CATS)
    order, blocks, ipad, slot_col = _layout(cat_sizes)

    rank = np.full(NUM_CATS, -1, np.int64)
    rank[order] = np.arange(len(order))

    # position of each item within its category (stable order)
    perm = np.argsort(cat, kind="stable")
    starts = np.searchsorted(cat[perm], np.arange(NUM_CATS))
    within_sorted = np.arange(NUM_ITEMS) - starts[cat[perm]]
    item_within = np.empty(NUM_ITEMS, np.int64)
    item_within[perm] = within_sorted

    r = rank[cat]
    item_shard = r % NCORES
    item_col = slot_col[r // NCORES] + item_within

    alpha = np.ascontiguousarray(np.asarray(inputs["alpha_item"], np.float32))
    obs = np.ascontiguousarray(np.asarray(inputs["item_obs"], np.float32))
    lam = np.asarray(inputs["lambda_item"], np.float32).ravel()

    W = np.zeros((NCORES, 2 * LATENT, ipad), np.float32)
    LAM = np.full((NCORES, 1, ipad), PAD_NEG, np.float32)
    for s in range(NCORES):
        m = item_shard == s
        cols = item_col[m]
        W[s, 0:LATENT, cols] = alpha[m]
        W[s, LATENT:, cols] = obs[m]
        LAM[s, 0, cols] = lam[m]
    W = W.astype(ml_dtypes.bfloat16)
    LAM = LAM.astype(ml_dtypes.bfloat16)

    uidx = np.asarray(inputs["user_index"]).astype(np.int64).ravel()
    theta = np.asarray(inputs["theta_user"], np.float32)
    zeta = np.asarray(inputs["zeta_user"], np.float32)
    # [2K, B] pre-gathered, pre-transposed session features (host-side shard prep)
    thzet = np.ascontiguousarray(
        np.concatenate([theta[uidx], zeta[uidx]], axis=1).T
    ).astype(ml_dtypes.bfloat16)
    return {
        "blocks": blocks,
        "ipad": ipad,
        "item_shard": item_shard,
        "item_col": item_col,
        "W": W,
        "LAM": LAM,
        "thzet": thzet,
    }


# ----------------------------------------------------------------------------
# Device program
# ----------------------------------------------------------------------------

def _bcast3(t2d, L):
    """[P, g] tile -> [P, g, L] read-AP with step-0 innermost broadcast."""
    ap = t2d[:, :]
    return bass.AP(tensor=ap.tensor, offset=ap.offset, ap=[*ap.ap, [0, L]])


def _build_nc(blocks, ipad):
    nc = bacc.Bacc(
        "TRN2",
        debug=False,
        enable_asserts=False,
        target_bir_lowering=False,
        num_devices=NCORES,
    )
    w_d = nc.dram_tensor("W", [2 * LATENT, ipad], BF16, kind="ExternalInput").ap()
    lam_d = nc.dram_tensor("LAM", [1, ipad], BF16, kind="ExternalInput").ap()
    thzet_d = nc.dram_tensor("THZET", [2 * LATENT, BATCH], BF16, kind="ExternalInput").ap()
    out_d = nc.dram_tensor("O", [BATCH, ipad], F16, kind="ExternalOutput").ap()

    gtot = sum(g for (_c, g, _l, _s) in blocks)
    with tile.TileContext(nc) as tc:
        with (
            tc.tile_pool(name="singles", bufs=1) as singles,
            tc.tile_pool(name="psum_u", bufs=2, space="PSUM") as psum_u,
            tc.tile_pool(name="ubuf", bufs=16) as ubuf,
            tc.tile_pool(name="exbuf", bufs=6) as exbuf,
            tc.tile_pool(name="stats", bufs=6) as stats,
        ):
            # DMA order: first block's W + lambda first, then the first two
            # session chunks, then the rest -- so the first matmul->evict
            # chain starts as early as possible.
            thzet_sb = singles.tile([2 * LATENT, BATCH], BF16, name="thzet_sb")
            w_sb = singles.tile([2 * LATENT, ipad], BF16, name="w_sb")
            lam128 = singles.tile([P, ipad], BF16, name="lam128")

            def load_block(col0, cols):
                nc.sync.dma_start(
                    out=w_sb[:, col0:col0 + cols], in_=w_d[:, col0:col0 + cols]
                )
                nc.sync.dma_start(
                    out=lam128[:, col0:col0 + cols],
                    in_=bass.AP(
                        tensor=lam_d.tensor, offset=col0, ap=[[0, P], [1, cols]]
                    ),
                )

            (c0_0, g_0, L_0, _s) = blocks[0]
            load_block(c0_0, g_0 * L_0)
            nc.sync.dma_start(
                out=thzet_sb[:, 0:2 * P], in_=thzet_d[:, 0:2 * P]
            )
            for (col0, g, L, _s0) in blocks[1:]:
                load_block(col0, g * L)
            nc.sync.dma_start(
                out=thzet_sb[:, 2 * P:], in_=thzet_d[:, 2 * P:]
            )
            thze_t = [thzet_sb[:, j * P:(j + 1) * P] for j in range(NCHUNKS)]

            # session-chunk groups: full JGROUP-sized groups up front, then
            # single-chunk groups at the end so the kernel tail (ln -> final
            # -> DMA of the last group) is as short as possible
            groups = []
            j = 0
            while j < NCHUNKS:
                n = JGROUP if j < NCHUNKS - 2 else 1
                groups.append(list(range(j, j + n)))
                j += n
            for gi, js in enumerate(groups):
                tail_group = gi >= len(groups) - 2
                u1s = {}
                s_g = stats.tile([P, len(js) * gtot], F32, name="s_g", tag="s_g")
                for t, j in enumerate(js):
                    goff = t * gtot
                    for bi, (col0, g, L, _s0) in enumerate(blocks):
                        cols = g * L
                        up = psum_u.tile([P, cols], F32, name="up", tag="up")
                        for c0 in range(0, cols, 512):
                            cn = min(512, cols - c0)
                            nc.tensor.matmul(
                                up[:, c0:c0 + cn],
                                lhsT=thze_t[j],
                                rhs=w_sb[:, col0 + c0:col0 + c0 + cn],
                                start=True,
                                stop=True,
                            )
                        # evict PSUM: u1 = u + lambda  (no exp-shift needed:
                        # |u1| < ~90 so exp stays inside f32 range, and ln(s)
                        # is computed exactly via exponent extraction)
                        u1 = ubuf.tile([P, cols], F32, name="u1", tag="u1")
                        nc.vector.scalar_tensor_tensor(
                            out=u1[:, :],
                            in0=up[:, :],
                            scalar=0.0,
                            in1=lam128[:, col0:col0 + cols],
                            op0=mybir.AluOpType.add,
                            op1=mybir.AluOpType.add,
                        )
                        u1s[(t, bi)] = u1
                        ex = exbuf.tile([P, cols], BF16, name="ex", tag="ex")
                        nc.scalar.activation(
                            out=ex[:, :], in_=u1[:, :],
                            func=mybir.ActivationFunctionType.Exp,
                        )
                        nc.vector.reduce_sum(
                            out=s_g[:, goff:goff + g],
                            in_=ex[:, :].rearrange("p (g l) -> p g l", l=L),
                            axis=mybir.AxisListType.X,
                        )
                        goff += g
                # ls = ln(s) via exponent extraction: s = 2^e * m, m in [1,2):
                # ln(s) = Ln_table(m) + (e-127)*ln2. Keeps the Ln table input
                # in its accurate range regardless of s's magnitude.
                gw = len(js) * gtot
                si = s_g[:, :].bitcast(mybir.dt.int32)
                eexp = stats.tile([P, gw], mybir.dt.int32, name="eexp", tag="eexp")
                nc.vector.tensor_scalar(
                    out=eexp[:, :], in0=si, scalar1=23, scalar2=None,
                    op0=mybir.AluOpType.logical_shift_right,
                )
                mbits = stats.tile([P, gw], mybir.dt.int32, name="mbits", tag="mbits")
                nc.vector.tensor_scalar(
                    out=mbits[:, :], in0=si,
                    scalar1=0x007FFFFF, scalar2=0x3F800000,
                    op0=mybir.AluOpType.bitwise_and,
                    op1=mybir.AluOpType.bitwise_or,
                )
                # e2 = (e - 127) * ln2 on ACT (Copy: out = in*scale + bias)
                e2 = stats.tile([P, gw], F32, name="e2", tag="e2")
                nc.scalar.activation(
                    out=e2[:, :], in_=eexp[:, :],
                    func=mybir.ActivationFunctionType.Copy,
                    bias=float(-127.0 * LN2),
                    scale=float(LN2),
                )
                lnm = stats.tile([P, gw], F32, name="lnm", tag="lnm")
                nc.scalar.activation(
                    out=lnm[:, :], in_=mbits[:, :].bitcast(F32),
                    func=mybir.ActivationFunctionType.Ln,
                )
                ls_g = stats.tile([P, gw], F32, name="ls_g", tag="ls_g")
                nc.vector.tensor_add(out=ls_g[:, :], in0=lnm[:, :], in1=e2[:, :])
                for t, j in enumerate(js):
                    goff = t * gtot
                    feng = nc.gpsimd
                    for bi, (col0, g, L, _s0) in enumerate(blocks):
                        cols = g * L
                        u1 = u1s[(t, bi)]
                        ls_b = _bcast3(ls_g[:, goff:goff + g], L)
                        # fp16 result written in place over u1's low half:
                        # out elem i lands behind the (monotonic) read cursor
                        # of in elem i, so the overlap is safe.
                        o16 = u1[:, :].bitcast(F16)[:, 0:cols]
                        if tail_group and bi == 1:
                            # tail: VectorE (idle by now) takes the big
                            # middle block so GpSimd's serial tail shrinks
                            nc.vector.scalar_tensor_tensor(
                                out=o16.rearrange("p (g l) -> p g l", l=L),
                                in0=u1[:, :].rearrange("p (g l) -> p g l", l=L),
                                scalar=0.0,
                                in1=ls_b,
                                op0=mybir.AluOpType.add,
                                op1=mybir.AluOpType.subtract,
                            )
                        else:
                            feng.tensor_tensor(
                                out=o16.rearrange("p (g l) -> p g l", l=L),
                                in0=u1[:, :].rearrange("p (g l) -> p g l", l=L),
                                in1=ls_b,
                                op=mybir.AluOpType.subtract,
                            )
                        nc.sync.dma_start(
                            out=out_d[j * P:(j + 1) * P, col0:col0 + cols],
                            in_=o16,
                        )
                        goff += g
    nc.compile()
    return nc


# ----------------------------------------------------------------------------
# Entry points
# ----------------------------------------------------------------------------

def run(inputs, trace=False):
    prep = _prep(inputs)
    key = (prep["ipad"], tuple(prep["blocks"]))
    nc = _nc_cache.get(key)
    if nc is None:
        nc = _build_nc(prep["blocks"], prep["ipad"])
        _nc_cache[key] = nc
    in_maps = [
        {
            "W": prep["W"][c],
            "LAM": prep["LAM"][c],
            "THZET": prep["thzet"],
        }
        for c in range(NCORES)
    ]
    res = bass_utils.run_bass_kernel_spmd(
        nc, in_maps, core_ids=list(range(NCORES)), trace=trace
    )
    big = np.stack([res.results[c]["O"] for c in range(NCORES)])  # [8, B, ipad] f16
    out = np.ascontiguousarray(
        big[prep["item_shard"], :, prep["item_col"]].T
    ).astype(np.float32)
    return out, res


def kernel(**inputs) -> np.ndarray:
    out, _ = run(inputs, trace=False)
    return out


# revision 17
# speedup vs baseline: 1.1780x; 1.1645x over previous
"""Trainium2 Bass kernel for nn_BEMBFlex (within-category log-softmax utility model).

Strategy: shard ITEMS BY CATEGORY across the 8 cores. Categories are
rank-sorted by size and dealt round-robin (rank % 8 -> shard), so every
shard receives the same padded column layout and one SPMD program serves
all cores. Each core computes util = [th|ze] @ [alphaT; item_obsT] for all
1024 sessions over its ~1/8 of the items (lambda folded in during the PSUM
eviction via a DMA-broadcast tile), then does the within-category
log-softmax locally (categories never span shards). Output leaves the
device as fp16 (halves the dominant out-DMA); the host de-permutes the
padded columns and upcasts to f32.
"""

import sys

for _p in ("/opt/trn_rl_repo",):
    if _p not in sys.path:
        sys.path.insert(0, _p)

import ml_dtypes
import numpy as np

import concourse.bass as bass
import concourse.tile as tile
from concourse import bacc, bass_utils, mybir

# Force Exp/Ln/Copy onto the one ACT table set that holds all three
# (natural_log_exp_and_others), so the kernel pays a single table load
# instead of reloading on every Exp<->Ln alternation. Entries keep their
# positions (set ids are positional); other sets merely lose the funcs
# that the combined set provides.
_orig_gat = bacc.get_activation_tables


def _gat_forced(arch):
    t = _orig_gat(arch)
    target = "natural_log_exp_and_others"
    if target in t:
        t = {k: (v if k == target else v - t[target]) for k, v in t.items()}
    return t


bacc.get_activation_tables = _gat_forced

NUM_USERS = 100000
NUM_ITEMS = 25000
NUM_CATS = 500
LATENT = 64
BATCH = 1024
NCORES = 8
P = 128                    # partitions / sessions per matmul chunk
NCHUNKS = BATCH // P       # session chunks per core
BLOCK_COLS = 2048          # max padded cols per processing block (4 PSUM banks)
JGROUP = 2                 # session-chunks sharing one ln batch
PAD_NEG = -1.0e30
LN2 = 0.6931471805599453

F32 = mybir.dt.float32
F16 = mybir.dt.float16
BF16 = mybir.dt.bfloat16
I32 = mybir.dt.int32

_nc_cache = {}


# ----------------------------------------------------------------------------
# Host-side layout
# ----------------------------------------------------------------------------

def _layout(cat_sizes):
    """Slot/block layout shared by all 8 shards.

    Categories sorted by size desc; slot i holds category ranks
    [8i, 8i+8) (one per shard). Slot width L_i = first (largest) size in
    the group rounded up to a multiple of 4. Blocks greedily group
    consecutive slots under a uniform L (the first slot's L) with
    g*L <= BLOCK_COLS.
    """
    order = np.argsort(-cat_sizes, kind="stable")
    order = order[cat_sizes[order] > 0]
    ncats = len(order)
    nslots = -(-ncats // NCORES)
    slot_L = np.empty(nslots, np.int64)
    for i in range(nslots):
        mx = int(cat_sizes[order[i * NCORES]])
        slot_L[i] = max(4, ((mx + 3) // 4) * 4)
    blocks = []  # (col0, g, L, slot0)
    col = 0
    i = 0
    while i < nslots:
        Lb = int(slot_L[i])
        # keep the first block small so the first matmul->evict chain
        # starts as early as possible
        cap = 512 if not blocks else BLOCK_COLS
        g = 1
        while i + g < nslots and (g + 1) * Lb <= cap:
            g += 1
        blocks.append((col, g, Lb, i))
        col += g * Lb
        i += g
    ipad = col
    slot_col = np.empty(nslots, np.int64)
    for (c0, g, Lb, s0) in blocks:
        for q in range(g):
            slot_col[s0 + q] = c0 + q * Lb
    return order, blocks, ipad, slot_col


def _prep(inputs):
    cat = np.asarray(inputs["category_idx"]).astype(np.int64).ravel()
    cat_sizes = np.bincount(cat, minlength=NUM_CATS)
    order, blocks, ipad, slot_col = _layout(cat_sizes)

    rank = np.full(NUM_CATS, -1, np.int64)
    rank[order] = np.arange(len(order))

    # position of each item within its category (stable order)
    perm = np.argsort(cat, kind="stable")
    starts = np.searchsorted(cat[perm], np.arange(NUM_CATS))
    within_sorted = np.arange(NUM_ITEMS) - starts[cat[perm]]
    item_within = np.empty(NUM_ITEMS, np.int64)
    item_within[perm] = within_sorted

    r = rank[cat]
    item_shard = r % NCORES
    item_col = slot_col[r // NCORES] + item_within

    alpha = np.ascontiguousarray(np.asarray(inputs["alpha_item"], np.float32))
    obs = np.ascontiguousarray(np.asarray(inputs["item_obs"], np.float32))
    lam = np.asarray(inputs["lambda_item"], np.float32).ravel()

    W = np.zeros((NCORES, 2 * LATENT, ipad), np.float32)
    LAM = np.full((NCORES, 1, ipad), PAD_NEG, np.float32)
    for s in range(NCORES):
        m = item_shard == s
        cols = item_col[m]
        W[s, 0:LATENT, cols] = alpha[m]
        W[s, LATENT:, cols] = obs[m]
        LAM[s, 0, cols] = lam[m]
    W = W.astype(ml_dtypes.bfloat16)
    LAM = LAM.astype(ml_dtypes.bfloat16)

    uidx = np.asarray(inputs["user_index"]).astype(np.int64).ravel()
    theta = np.asarray(inputs["theta_user"], np.float32)
    zeta = np.asarray(inputs["zeta_user"], np.float32)
    # [2K, B] pre-gathered, pre-transposed session features (host-side shard prep)
    thzet = np.ascontiguousarray(
        np.concatenate([theta[uidx], zeta[uidx]], axis=1).T
    ).astype(ml_dtypes.bfloat16)
    return {
        "blocks": blocks,
        "ipad": ipad,
        "item_shard": item_shard,
        "item_col": item_col,
        "W": W,
        "LAM": LAM,
        "thzet": thzet,
    }


# ----------------------------------------------------------------------------
# Device program
# ----------------------------------------------------------------------------

def _bcast3(t2d, L):
    """[P, g] tile -> [P, g, L] read-AP with step-0 innermost broadcast."""
    ap = t2d[:, :]
    return bass.AP(tensor=ap.tensor, offset=ap.offset, ap=[*ap.ap, [0, L]])


def _build_nc(blocks, ipad):
    nc = bacc.Bacc(
        "TRN2",
        debug=False,
        enable_asserts=False,
        target_bir_lowering=False,
        num_devices=NCORES,
    )
    w_d = nc.dram_tensor("W", [2 * LATENT, ipad], BF16, kind="ExternalInput").ap()
    lam_d = nc.dram_tensor("LAM", [1, ipad], BF16, kind="ExternalInput").ap()
    thzet_d = nc.dram_tensor("THZET", [2 * LATENT, BATCH], BF16, kind="ExternalInput").ap()
    out_d = nc.dram_tensor("O", [BATCH, ipad], F16, kind="ExternalOutput").ap()

    gtot = sum(g for (_c, g, _l, _s) in blocks)
    with tile.TileContext(nc) as tc:
        with (
            tc.tile_pool(name="singles", bufs=1) as singles,
            tc.tile_pool(name="psum_u", bufs=2, space="PSUM") as psum_u,
            tc.tile_pool(name="ubuf", bufs=16) as ubuf,
            tc.tile_pool(name="exbuf", bufs=6) as exbuf,
            tc.tile_pool(name="stats", bufs=6) as stats,
        ):
            # DMA order: first block's W + lambda first, then the first two
            # session chunks, then the rest -- so the first matmul->evict
            # chain starts as early as possible.
            thzet_sb = singles.tile([2 * LATENT, BATCH], BF16, name="thzet_sb")
            w_sb = singles.tile([2 * LATENT, ipad], BF16, name="w_sb")
            lam128 = singles.tile([P, ipad], BF16, name="lam128")

            def load_block(col0, cols):
                nc.sync.dma_start(
                    out=w_sb[:, col0:col0 + cols], in_=w_d[:, col0:col0 + cols]
                )
                nc.sync.dma_start(
                    out=lam128[:, col0:col0 + cols],
                    in_=bass.AP(
                        tensor=lam_d.tensor, offset=col0, ap=[[0, P], [1, cols]]
                    ),
                )

            (c0_0, g_0, L_0, _s) = blocks[0]
            load_block(c0_0, g_0 * L_0)
            nc.sync.dma_start(
                out=thzet_sb[:, 0:2 * P], in_=thzet_d[:, 0:2 * P]
            )
            for (col0, g, L, _s0) in blocks[1:]:
                load_block(col0, g * L)
            nc.sync.dma_start(
                out=thzet_sb[:, 2 * P:], in_=thzet_d[:, 2 * P:]
            )
            thze_t = [thzet_sb[:, j * P:(j + 1) * P] for j in range(NCHUNKS)]

            # session-chunk groups: full JGROUP-sized groups up front, then
            # single-chunk groups at the end so the kernel tail (ln -> final
            # -> DMA of the last group) is as short as possible
            groups = []
            j = 0
            while j < NCHUNKS:
                n = JGROUP if j < NCHUNKS - 2 else 1
                groups.append(list(range(j, j + n)))
                j += n
            for js in groups:
                u1s = {}
                s_g = stats.tile([P, len(js) * gtot], F32, name="s_g", tag="s_g")
                for t, j in enumerate(js):
                    goff = t * gtot
                    for bi, (col0, g, L, _s0) in enumerate(blocks):
                        cols = g * L
                        up = psum_u.tile([P, cols], F32, name="up", tag="up")
                        for c0 in range(0, cols, 512):
                            cn = min(512, cols - c0)
                            nc.tensor.matmul(
                                up[:, c0:c0 + cn],
                                lhsT=thze_t[j],
                                rhs=w_sb[:, col0 + c0:col0 + c0 + cn],
                                start=True,
                                stop=True,
                            )
                        # evict PSUM: u1 = u + lambda  (no exp-shift needed:
                        # |u1| < ~90 so exp stays inside f32 range, and ln(s)
                        # is computed exactly via exponent extraction)
                        u1 = ubuf.tile([P, cols], F32, name="u1", tag="u1")
                        nc.vector.scalar_tensor_tensor(
                            out=u1[:, :],
                            in0=up[:, :],
                            scalar=0.0,
                            in1=lam128[:, col0:col0 + cols],
                            op0=mybir.AluOpType.add,
                            op1=mybir.AluOpType.add,
                        )
                        u1s[(t, bi)] = u1
                        ex = exbuf.tile([P, cols], BF16, name="ex", tag="ex")
                        nc.scalar.activation(
                            out=ex[:, :], in_=u1[:, :],
                            func=mybir.ActivationFunctionType.Exp,
                        )
                        nc.vector.reduce_sum(
                            out=s_g[:, goff:goff + g],
                            in_=ex[:, :].rearrange("p (g l) -> p g l", l=L),
                            axis=mybir.AxisListType.X,
                        )
                        goff += g
                # ls = ln(s) via exponent extraction: s = 2^e * m, m in [1,2):
                # ln(s) = Ln_table(m) + (e-127)*ln2. Keeps the Ln table input
                # in its accurate range regardless of s's magnitude.
                gw = len(js) * gtot
                si = s_g[:, :].bitcast(mybir.dt.int32)
                eexp = stats.tile([P, gw], mybir.dt.int32, name="eexp", tag="eexp")
                nc.vector.tensor_scalar(
                    out=eexp[:, :], in0=si, scalar1=23, scalar2=None,
                    op0=mybir.AluOpType.logical_shift_right,
                )
                mbits = stats.tile([P, gw], mybir.dt.int32, name="mbits", tag="mbits")
                nc.vector.tensor_scalar(
                    out=mbits[:, :], in0=si,
                    scalar1=0x007FFFFF, scalar2=0x3F800000,
                    op0=mybir.AluOpType.bitwise_and,
                    op1=mybir.AluOpType.bitwise_or,
                )
                # e2 = (e - 127) * ln2 on ACT (Copy: out = in*scale + bias)
                e2 = stats.tile([P, gw], F32, name="e2", tag="e2")
                nc.scalar.activation(
                    out=e2[:, :], in_=eexp[:, :],
                    func=mybir.ActivationFunctionType.Copy,
                    bias=float(-127.0 * LN2),
                    scale=float(LN2),
                )
                lnm = stats.tile([P, gw], F32, name="lnm", tag="lnm")
                nc.scalar.activation(
                    out=lnm[:, :], in_=mbits[:, :].bitcast(F32),
                    func=mybir.ActivationFunctionType.Ln,
                )
                ls_g = stats.tile([P, gw], F32, name="ls_g", tag="ls_g")
                nc.vector.tensor_add(out=ls_g[:, :], in0=lnm[:, :], in1=e2[:, :])
                for t, j in enumerate(js):
                    goff = t * gtot
                    feng = nc.gpsimd
                    for bi, (col0, g, L, _s0) in enumerate(blocks):
                        cols = g * L
                        u1 = u1s[(t, bi)]
                        ls_b = _bcast3(ls_g[:, goff:goff + g], L)
                        # fp16 result written in place over u1's low half:
                        # out elem i lands behind the (monotonic) read cursor
                        # of in elem i, so the overlap is safe.
                        o16 = u1[:, :].bitcast(F16)[:, 0:cols]
                        feng.tensor_tensor(
                            out=o16.rearrange("p (g l) -> p g l", l=L),
                            in0=u1[:, :].rearrange("p (g l) -> p g l", l=L),
                            in1=ls_b,
                            op=mybir.AluOpType.subtract,
                        )
                        nc.sync.dma_start(
                            out=out_d[j * P:(j + 1) * P, col0:col0 + cols],
                            in_=o16,
                        )
                        goff += g
    nc.compile()
    return nc


# ----------------------------------------------------------------------------
# Entry points
# ----------------------------------------------------------------------------

def run(inputs, trace=False):
    prep = _prep(inputs)
    key = (prep["ipad"], tuple(prep["blocks"]))
    nc = _nc_cache.get(key)
    if nc is None:
        nc = _build_nc(prep["blocks"], prep["ipad"])
        _nc_cache[key] = nc
    in_maps = [
        {
            "W": prep["W"][c],
            "LAM": prep["LAM"][c],
            "THZET": prep["thzet"],
        }
        for c in range(NCORES)
    ]
    res = bass_utils.run_bass_kernel_spmd(
        nc, in_maps, core_ids=list(range(NCORES)), trace=trace
    )
    big = np.stack([res.results[c]["O"] for c in range(NCORES)])  # [8, B, ipad] f16
    out = np.ascontiguousarray(
        big[prep["item_shard"], :, prep["item_col"]].T
    ).astype(np.float32)
    return out, res


def kernel(**inputs) -> np.ndarray:
    out, _ = run(inputs, trace=False)
    return out
